# revision 47
# baseline (speedup 1.0000x reference)
"""Trainium2 Bass kernel for nn_Block (attention + noisy top-2 MoE block).

Sharding (8 NeuronCores):
  Launch 1: core c -> batch b=c//2, head-group hg=c%2 (4 of 8 heads).
    LN1 -> q/k/v (local heads) -> causal softmax attention -> Wo partial,
    pairwise ReduceScatter over (hg0, hg1) -> each core owns 1024 tokens:
    residual + LN2 + noisy gating logits.
  Host: top-2 routing + capacity assignment (metadata only).
  Launch 2: core e -> expert e. Indirect-gather its CAP tokens, FFN
    (relu(x@W1+b1)@W2+b2) in bf16, scale by gate, return w_e.
  Host: out = h1; out[kept_e] += w_e  (indices unique within an expert).
"""

import numpy as np
import ml_dtypes

import concourse.bacc as bacc
import concourse.bass as bass
import concourse.mybir as mybir
import concourse.tile as tile
from concourse.bass_utils import run_bass_kernel_spmd
from concourse.masks import make_identity

B, T, D, H, HS = 4, 2048, 512, 8, 64
E, TOPK, CAP = 8, 2, 2048
N = B * T
EPS = 1e-5

FP = mybir.dt.float32
BF = mybir.dt.bfloat16
FR = mybir.dt.float32r
I32 = mybir.dt.int32
NP_BF = ml_dtypes.bfloat16

# dtype config: attention compute dtypes (routing-sensitive), FFN is bf16.
CFG = {
    "s_dt": FR,    # q/k and S=q@kT matmul dtype (f32r: fp32-precision, 1.5 cyc/row)
    "av_dt": FR,   # exp(S), v, AV matmul dtype
    "wo_dt": FR,   # attn^T / Wo matmul dtype
    "h2_dt": BF,   # h2 storage (expert gather table)
}

_NP_OF = {FP: np.float32, BF: NP_BF, FR: np.float32}


# --------------------------------------------------------------------------
# Launch 1: attention + gating logits
# --------------------------------------------------------------------------

def build_l1(cfg=CFG, no_collective=False):
    s_dt, av_dt, wo_dt, h2_dt = cfg["s_dt"], cfg["av_dt"], cfg["wo_dt"], cfg["h2_dt"]
    x_dt = s_dt if s_dt == av_dt else FP
    nc = bacc.Bacc(num_devices=8)

    xb = nc.dram_tensor("xb", [T, D], FP, kind="ExternalInput")
    xown = nc.dram_tensor("xown", [T // 2, D], FP, kind="ExternalInput")
    wq = nc.dram_tensor("wq", [D, 256], x_dt, kind="ExternalInput")
    wk = nc.dram_tensor("wk", [D, 256], x_dt, kind="ExternalInput")
    wv = nc.dram_tensor("wv", [D, 256], x_dt, kind="ExternalInput")
    qb = nc.dram_tensor("qb", [128, 2], FP, kind="ExternalInput")
    kb = nc.dram_tensor("kb", [128, 2], FP, kind="ExternalInput")
    wo = nc.dram_tensor("wo", [256, D], wo_dt, kind="ExternalInput")
    bo = nc.dram_tensor("bo", [D], FP, kind="ExternalInput")
    masks = nc.dram_tensor("masks", [128, 128], av_dt, kind="ExternalInput")
    wgn = nc.dram_tensor("wgn", [D, 64], FP, kind="ExternalInput")
    bgn = nc.dram_tensor("bgn", [64, 1], FP, kind="ExternalInput")

    h1o = nc.dram_tensor("h1o", [T // 2, D], FP, kind="ExternalOutput")
    h2o = nc.dram_tensor("h2o", [T // 2, D], h2_dt, kind="ExternalOutput")
    lgt = nc.dram_tensor("lgt", [16, T // 2], FP, kind="ExternalOutput")

    ccs = [nc.dram_tensor(f"cc{j}", [512, D], FP) for j in range(4)]
    rss = [nc.dram_tensor(f"rs{j}", [256, D], FP) for j in range(4)]

    with tile.TileContext(nc) as tc:
        with (
            tc.tile_pool(name="const", bufs=1) as const,
            tc.tile_pool(name="big", bufs=1) as big,
            tc.tile_pool(name="xtc", bufs=2) as xtc_pool,
            tc.tile_pool(name="qtc", bufs=2) as qtc_pool,
            tc.tile_pool(name="tmp", bufs=3) as tmp,
            tc.tile_pool(name="pexp", bufs=6) as pexp_pool,
            tc.tile_pool(name="rows", bufs=2) as rows,
            tc.tile_pool(name="epool", bufs=2) as epool,
            tc.tile_pool(name="small", bufs=3) as small,
            tc.tile_pool(name="psA", bufs=2, space="PSUM") as psA,
            tc.tile_pool(name="psS", bufs=3, space="PSUM") as psS,
            tc.tile_pool(name="psAV", bufs=1, space="PSUM") as psAV,
            tc.tile_pool(name="psT", bufs=1, space="PSUM") as psT,
        ):
            # ---- constants ----
            ident = const.tile([128, 128], FP)
            make_identity(nc, ident[:])
            tr_dt = BF if x_dt == BF else FP
            if tr_dt == FP:
                identx = ident
            else:
                identx = const.tile([128, 128], tr_dt)
                make_identity(nc, identx[:])
            zero = const.tile([128, 1], FP)
            nc.vector.memset(zero[:], 0.0)
            eps = const.tile([128, 1], FP)
            nc.vector.memset(eps[:], EPS)

            wq_sb = const.tile([128, 4, 256], x_dt)
            nc.scalar.dma_start(wq_sb[:], wq[:].rearrange("(ko p) m -> p ko m", p=128))
            wk_sb = const.tile([128, 4, 256], x_dt)
            nc.scalar.dma_start(wk_sb[:], wk[:].rearrange("(ko p) m -> p ko m", p=128))
            wv_sb = const.tile([128, 4, 256], x_dt)
            nc.scalar.dma_start(wv_sb[:], wv[:].rearrange("(ko p) m -> p ko m", p=128))
            wo_sb = const.tile([128, 2, 512], wo_dt)
            nc.scalar.dma_start(wo_sb[:], wo[:].rearrange("(ko p) m -> p ko m", p=128))
            wgn_sb = const.tile([128, 4, 64], FP)
            nc.scalar.dma_start(wgn_sb[:], wgn[:].rearrange("(ko p) m -> p ko m", p=128))
            bgn_sb = const.tile([64, 1], FP)
            nc.sync.dma_start(bgn_sb[:], bgn[:])
            qb_sb = const.tile([128, 2], FP)
            nc.sync.dma_start(qb_sb[:], qb[:])
            kb_sb = const.tile([128, 2], FP)
            nc.sync.dma_start(kb_sb[:], kb[:])

            def bcast_from_dram(dram, n):
                t = const.tile([128, n], FP, tag=f"bc_{dram.name}")
                src = bass.AP(tensor=dram, offset=0, ap=[[0, 128], [1, n]])
                nc.sync.dma_start(t[:], src)
                return t

            bo_bc = bcast_from_dram(bo, 512)

            masks_sb = const.tile([128, 128], av_dt)
            nc.scalar.dma_start(masks_sb[:], masks[:])

            # ---- persistent activations ----
            kT = big.tile([128, 2, T], s_dt)            # [hs-in-pair, head-pair, tok]
            v_aug = big.tile([128, 16, 4, 65], av_dt)   # [tok%128, tokblk, head, hs+1]
            onesf = const.tile([128, 16, 4, 1], FP)
            nc.vector.memset(onesf[:], 1.0)
            nc.vector.tensor_copy(v_aug[:, :, :, 64:65], onesf[:])

            # ---- LN1 stats prologue: ACT does all Sqrts before the Exps ----
            sqrt_insts = []
            last_exp = [None]
            mvs = const.tile([128, 16, 2], FP)
            rstds = const.tile([128, 16], FP)
            for j in range(4):
                xq = xtc_pool.tile([128, 4, 512], FP, tag="xq")
                nc.sync.dma_start(
                    xq[:], xb[j * 512:(j + 1) * 512, :].rearrange("(o p) m -> p o m", p=128))
                for t4 in range(4):
                    t = 4 * j + t4
                    stats = small.tile([128, 6], FP, tag="stats")
                    nc.vector.bn_stats(stats[:], xq[:, t4, :])
                    nc.vector.bn_aggr(mvs[:, t, :], stats[:])
                    sqrt_insts.append(nc.scalar.activation(
                        rstds[:, t:t + 1], mvs[:, t, 1:2],
                        mybir.ActivationFunctionType.Sqrt,
                        bias=eps[:], scale=1.0))
                    nc.vector.reciprocal(rstds[:, t:t + 1], rstds[:, t:t + 1])

            # ---- main loop over 512-token chunks ----
            for ch in range(4):
                # LN1 apply + x^T
                xTc = xtc_pool.tile([128, 4, 512], x_dt)
                xq2 = xtc_pool.tile([128, 4, 512], FP, tag="xq")
                nc.sync.dma_start(
                    xq2[:], xb[ch * 512:(ch + 1) * 512, :].rearrange("(o p) m -> p o m", p=128))
                for it4 in range(4):
                    t = ch * 4 + it4
                    lnx = tmp.tile([128, 512], FP, tag="w")
                    nc.vector.tensor_scalar(lnx[:], xq2[:, it4, :], scalar1=mvs[:, t, 0:1],
                                            scalar2=rstds[:, t:t + 1],
                                            op0=mybir.AluOpType.subtract,
                                            op1=mybir.AluOpType.mult)
                    if tr_dt != FP:
                        lnxc = tmp.tile([128, 512], tr_dt, tag="lnxb")
                        nc.vector.tensor_copy(lnxc[:], lnx[:])
                    else:
                        lnxc = lnx
                    pt = psT.tile([128, 512], tr_dt, tag="pt")
                    for db in range(4):
                        nc.tensor.transpose(pt[:, db * 128:(db + 1) * 128],
                                            lnxc[:, db * 128:(db + 1) * 128], identx[:])
                    nc.vector.tensor_copy(
                        xTc[:, :, it4 * 128:(it4 + 1) * 128],
                        pt[:].rearrange("p (a b) -> p a b", a=4))
                # q^T (chunk-local), k^T (persistent)
                qTc = qtc_pool.tile([128, 2, 512], s_dt)
                for hp in range(2):
                    pq = psA.tile([128, 512], FP, tag="pq", name="pq")
                    for db in range(4):
                        nc.tensor.matmul(pq[:], wq_sb[:, db, hp * 128:(hp + 1) * 128],
                                         xTc[:, db, :], start=db == 0, stop=db == 3)
                    nc.vector.tensor_scalar(qTc[:, hp, :], pq[:], scalar1=qb_sb[:, hp:hp + 1],
                                            scalar2=None, op0=mybir.AluOpType.add)
                    pk = psA.tile([128, 512], FP, tag="pq", name="pk")
                    for db in range(4):
                        nc.tensor.matmul(pk[:], wk_sb[:, db, hp * 128:(hp + 1) * 128],
                                         xTc[:, db, :], start=db == 0, stop=db == 3)
                    nc.vector.tensor_scalar(kT[:, hp, ch * 512:(ch + 1) * 512], pk[:],
                                            scalar1=kb_sb[:, hp:hp + 1],
                                            scalar2=None, op0=mybir.AluOpType.add)
                # v (row-major, augmented)
                for it4 in range(4):
                    t = ch * 4 + it4
                    pv = psA.tile([128, 512], FP, tag="pq", name="pv")
                    for db in range(4):
                        nc.tensor.matmul(pv[:, :256], xTc[:, db, it4 * 128:(it4 + 1) * 128],
                                         wv_sb[:, db, :], start=db == 0, stop=db == 3)
                    nc.vector.tensor_copy(
                        v_aug[:, t, :, 0:64],
                        pv[:, :256].rearrange("p (h e) -> p h e", h=4))
                # attention for this tq-chunk; heads of a pair row-packed on PE
                attnT = qtc_pool.tile([128, 2, 512], wo_dt, tag="attnT")
                for hp in range(2):
                    nvis = 4 * (ch + 1)
                    pavs = [psAV.tile([128, 512], FP, tag=f"pav{i}", name=f"pav{i}")
                            for i in range(2)]
                    for blk in range(nvis):
                        r = blk - 4 * ch
                        off = 128 * r if r > 0 else 0  # cols < off are fully masked
                        sss = []
                        for i in range(2):
                            base = i * 64
                            ss = psS.tile([128, 512], FP, tag="ss", name="ss")
                            nc.tensor.matmul(ss[:, off:],
                                             kT[base:base + 64, hp, blk * 128:(blk + 1) * 128],
                                             qTc[base:base + 64, hp, off:],
                                             start=True, stop=True,
                                             tile_position=(base, 0))
                            sss.append(ss)
                        pexps = []
                        for i in range(2):
                            pexp = pexp_pool.tile([128, 512], av_dt, tag="pexp", name="pexp")
                            exp_inst = nc.scalar.activation(
                                pexp[:, off:], sss[i][:, off:],
                                mybir.ActivationFunctionType.Exp,
                                bias=zero[:], scale=1.0)
                            last_exp[0] = exp_inst
                            if sqrt_insts is not None:
                                for si in sqrt_insts:
                                    tile.add_dep_helper(exp_inst.ins, si.ins, sync=False,
                                                        reason="act table grouping")
                                sqrt_insts = None
                            if r >= 0:
                                # only the 128-wide strip crossing the diagonal needs masking
                                nc.gpsimd.tensor_tensor(pexp[:, off:off + 128],
                                                        pexp[:, off:off + 128], masks_sb[:],
                                                        op=mybir.AluOpType.mult)
                            pexps.append(pexp)
                        for i in range(2):
                            nc.tensor.matmul(pavs[i][:65, off:], v_aug[:, blk, 2 * hp + i, :],
                                             pexps[i][:, off:],
                                             start=blk == 0, stop=blk == nvis - 1)
                    for i in range(2):
                        base = i * 64
                        rec = rows.tile([1, 512], FP, tag="rec", name="rec")
                        nc.vector.tensor_copy(rec[:], pavs[i][64:65, :])
                        nc.vector.reciprocal(rec[:], rec[:])
                        bc = rows.tile([64, 512], FP, tag="bc", name="bc")
                        nc.gpsimd.partition_broadcast(bc[:], rec[:])
                        nc.vector.tensor_tensor(attnT[base:base + 64, hp, :],
                                                pavs[i][0:64, :], bc[:],
                                                op=mybir.AluOpType.mult)
                # Wo for this tq-chunk + its pairwise reduce-scatter
                ho_big = qtc_pool.tile([128, 4, 512], FP, tag="hob", name="ho_big")
                for it4 in range(4):
                    po = psA.tile([128, 512], FP, tag="pq", name="po")
                    for kbk in range(2):
                        nc.tensor.matmul(po[:], attnT[:, kbk, it4 * 128:(it4 + 1) * 128],
                                         wo_sb[:, kbk, :], start=kbk == 0, stop=kbk == 1)
                    nc.vector.tensor_copy(ho_big[:, it4, :], po[:])
                nc.sync.dma_start(ccs[ch][:].rearrange("(o p) m -> p o m", p=128), ho_big[:])
                if no_collective:
                    half = big.tile([128, 2, 512], FP, tag="rshack", name="half")
                    nc.sync.dma_start(half[:], ccs[ch][:256, :].rearrange("(o p) m -> p o m", p=128))
                    nc.sync.dma_start(rss[ch][:].rearrange("(o p) m -> p o m", p=128), half[:])
                else:
                    nc.gpsimd.collective_compute(
                        "ReduceScatter", mybir.AluOpType.add,
                        replica_groups=[[0, 1], [2, 3], [4, 5], [6, 7]],
                        ins=[ccs[ch][:]], outs=[rss[ch][:]])

            # ---- stage E: residual + LN2 + h2^T ----
            h2T = big.tile([128, 4, T // 2], FP)
            for it in range(8):
                if it % 2 == 0:
                    rtb = epool.tile([128, 2, 512], FP, tag="rtb", name="rtb")
                    nc.sync.dma_start(
                        rtb[:], rss[it // 2][:].rearrange("(o p) m -> p o m", p=128))
                    xob = epool.tile([128, 2, 512], FP, tag="xob", name="xob")
                    nc.sync.dma_start(
                        xob[:], xown[it * 128:(it + 2) * 128, :].rearrange("(o p) m -> p o m", p=128))
                    h1st = epool.tile([128, 2, 512], FP, tag="h1st", name="h1st")
                    h2st = epool.tile([128, 2, 512], h2_dt, tag="h2st", name="h2st")
                h1t = h1st[:, it % 2, :]
                nc.gpsimd.tensor_tensor(h1t, rtb[:, it % 2, :], xob[:, it % 2, :],
                                        op=mybir.AluOpType.add)
                nc.gpsimd.tensor_tensor(h1t, h1t, bo_bc[:], op=mybir.AluOpType.add)
                stats = small.tile([128, 6], FP, tag="stats")
                nc.vector.bn_stats(stats[:], h1t)
                mv = small.tile([128, 2], FP, tag="mv")
                nc.vector.bn_aggr(mv[:], stats[:])
                rstd = small.tile([128, 1], FP, tag="rstd")
                e_sqrt = nc.scalar.activation(rstd[:], mv[:, 1:2],
                                              mybir.ActivationFunctionType.Sqrt,
                                              bias=eps[:], scale=1.0)
                if last_exp[0] is not None:
                    tile.add_dep_helper(e_sqrt.ins, last_exp[0].ins, sync=False,
                                        reason="act table grouping (ln2)")
                nc.vector.reciprocal(rstd[:], rstd[:])
                h2t = small.tile([128, 512], FP, tag="we")
                nc.vector.tensor_scalar(h2t[:], h1t, scalar1=mv[:, 0:1],
                                        scalar2=rstd[:],
                                        op0=mybir.AluOpType.subtract,
                                        op1=mybir.AluOpType.mult)
                nc.vector.tensor_copy(h2st[:, it % 2, :], h2t[:])
                pt2 = psT.tile([128, 512], FP, tag="pt")
                for db in range(4):
                    nc.tensor.transpose(pt2[:, db * 128:(db + 1) * 128],
                                        h2t[:, db * 128:(db + 1) * 128], ident[:])
                nc.vector.tensor_copy(
                    h2T[:, :, it * 128:(it + 1) * 128],
                    pt2[:].rearrange("p (a b) -> p a b", a=4))
                if it % 2 == 1:
                    j2 = it // 2
                    nc.sync.dma_start(
                        h1o[j2 * 256:(j2 + 1) * 256, :].rearrange("(o p) m -> p o m", p=128),
                        h1st[:])
                    nc.sync.dma_start(
                        h2o[j2 * 256:(j2 + 1) * 256, :].rearrange("(o p) m -> p o m", p=128),
                        h2st[:])

            # ---- stage F: gating (rows 0-7 logits, 32-39 noise logits) ----
            for c2 in range(2):
                pg = psA.tile([128, 512], FP, tag="pq", name="pg")
                for db in range(4):
                    nc.tensor.matmul(pg[:64, :], wgn_sb[:, db, :],
                                     h2T[:, db, c2 * 512:(c2 + 1) * 512],
                                     start=db == 0, stop=db == 3)
                nc.vector.tensor_scalar(pg[:64, :], pg[:64, :], scalar1=bgn_sb[:],
                                        scalar2=None, op0=mybir.AluOpType.add)
                t1 = rows.tile([8, 512], FP, tag="sp")
                nc.vector.tensor_copy(t1[:], pg[0:8, :])
                nc.sync.dma_start(lgt[0:8, c2 * 512:(c2 + 1) * 512], t1[:])
                t2 = rows.tile([8, 512], FP, tag="sp")
                nc.vector.tensor_copy(t2[:], pg[32:40, :])
                nc.sync.dma_start(lgt[8:16, c2 * 512:(c2 + 1) * 512], t2[:])

    nc.compile()
    return nc


# --------------------------------------------------------------------------
# Launch 2: expert FFN
# --------------------------------------------------------------------------

def build_l2(cfg=CFG):
    h2_dt = cfg["h2_dt"]
    nc = bacc.Bacc(num_devices=8)

    h2full = nc.dram_tensor("h2full", [N, D], h2_dt, kind="ExternalInput")
    idxb = nc.dram_tensor("idxb", [CAP, 1], I32, kind="ExternalInput")
    gv = nc.dram_tensor("gv", [CAP, 1], FP, kind="ExternalInput")
    w1 = nc.dram_tensor("w1", [D, 4 * D], BF, kind="ExternalInput")
    w2 = nc.dram_tensor("w2", [4 * D, D], BF, kind="ExternalInput")
    b1 = nc.dram_tensor("b1", [4 * D], FP, kind="ExternalInput")
    b2 = nc.dram_tensor("b2", [D], FP, kind="ExternalInput")
    wout = nc.dram_tensor("wout", [CAP, D], FP, kind="ExternalOutput")

    with tile.TileContext(nc) as tc:
        with (
            tc.tile_pool(name="const", bufs=1) as const,
            tc.tile_pool(name="chunk", bufs=2) as chunk,
            tc.tile_pool(name="tmp", bufs=3) as tmp,
            tc.tile_pool(name="psM", bufs=3, space="PSUM") as psM,
            tc.tile_pool(name="psO", bufs=2, space="PSUM") as psO,
        ):
            w1_sb = const.tile([128, 4, 4 * D], BF)
            nc.sync.dma_start(w1_sb[:], w1[:].rearrange("(ko p) m -> p ko m", p=128))
            w2_sb = const.tile([128, 16, D], BF)
            nc.scalar.dma_start(w2_sb[:], w2[:].rearrange("(ko p) m -> p ko m", p=128))
            idx_sb = const.tile([128, 16], I32)
            nc.sync.dma_start(idx_sb[:], idxb[:, 0].rearrange("(o p) -> p o", p=128))
            gv_sb = const.tile([128, 16], FP)
            nc.sync.dma_start(gv_sb[:], gv[:, 0].rearrange("(o p) -> p o", p=128))
            b1_sb = const.tile([128, 16], FP)
            nc.sync.dma_start(b1_sb[:], b1[:].rearrange("(o p) -> p o", p=128))
            b2_bc = const.tile([128, 512], FP)
            nc.sync.dma_start(b2_bc[:], bass.AP(tensor=b2, offset=0, ap=[[0, 128], [1, 512]]))

            for tc4 in range(4):  # 512-token chunks
                xgT = chunk.tile([128, 4, 512], h2_dt, tag="xgT")
                for it4 in range(4):
                    t = tc4 * 4 + it4
                    xg = tmp.tile([128, 512], h2_dt, tag="xg")
                    nc.gpsimd.indirect_dma_start(
                        out=xg[:], out_offset=None, in_=h2full[:],
                        in_offset=bass.IndirectOffsetOnAxis(ap=idx_sb[:, t:t + 1], axis=0))
                    for db in range(4):
                        nc.sync.dma_start_transpose(
                            xgT[:, db, it4 * 128:(it4 + 1) * 128],
                            xg[:, db * 128:(db + 1) * 128])
                midT = chunk.tile([128, 16, 512], BF, tag="midT")
                for fb in range(16):
                    pm = psM.tile([128, 512], FP, tag="pm")
                    for db in range(4):
                        nc.tensor.matmul(pm[:], w1_sb[:, db, fb * 128:(fb + 1) * 128],
                                         xgT[:, db, :], start=db == 0, stop=db == 3)
                    nc.scalar.activation(midT[:, fb, :], pm[:],
                                         mybir.ActivationFunctionType.Relu,
                                         bias=b1_sb[:, fb:fb + 1], scale=1.0)
                ost = chunk.tile([128, 4, 512], FP, tag="ost")
                for it4 in range(4):
                    t = tc4 * 4 + it4
                    po = psO.tile([128, 512], FP, tag="po")
                    for fb in range(16):
                        nc.tensor.matmul(po[:], midT[:, fb, it4 * 128:(it4 + 1) * 128],
                                         w2_sb[:, fb, :], start=fb == 0, stop=fb == 15)
                    ot = ost[:, it4, :]
                    nc.vector.tensor_tensor(ot, po[:], b2_bc[:], op=mybir.AluOpType.add)
                    nc.vector.tensor_scalar(ot, ot, scalar1=gv_sb[:, t:t + 1], scalar2=None,
                                            op0=mybir.AluOpType.mult)
                nc.sync.dma_start(
                    wout[tc4 * 512:(tc4 + 1) * 512, :].rearrange("(o p) m -> p o m", p=128),
                    ost[:])

    nc.compile()
    return nc


# --------------------------------------------------------------------------
# Host glue
# --------------------------------------------------------------------------

_BUILT = {}


def _own_idx(c):
    """Global token indices owned by core c, in its local row order."""
    b, hg = c // 2, c % 2
    return np.concatenate(
        [b * T + 512 * j + 256 * hg + np.arange(256) for j in range(4)])


def _get_built():
    if "l1" not in _BUILT:
        _BUILT["l1"] = build_l1()
        _BUILT["l2"] = build_l2()
    return _BUILT["l1"], _BUILT["l2"]


def _get_noise():
    # The reference adds jax.random.normal(key(42), [B,T,E]) noise to the
    # gating logits; threefry is deterministic, so precompute it host-side.
    if "noise" not in _BUILT:
        import jax
        import jax.numpy as jnp
        try:
            dev = jax.devices("cpu")[0]
        except Exception:
            dev = None
        if dev is not None:
            with jax.default_device(dev):
                noise = jax.random.normal(jax.random.key(42), (B, T, E), jnp.float32)
        else:
            noise = jax.random.normal(jax.random.key(42), (B, T, E), jnp.float32)
        _BUILT["noise"] = np.asarray(noise).reshape(N, E)
    return _BUILT["noise"]


def _build_masks(np_dt):
    j = np.arange(128)[None, :]
    p = np.arange(128)[:, None]
    return (j >= p).astype(np_dt)


def l1_in_maps(inp, cfg=CFG):
    s_np = _NP_OF[cfg["s_dt"]]
    av_np = _NP_OF[cfg["av_dt"]]
    wo_np = _NP_OF[cfg["wo_dt"]]
    x_np = _NP_OF[cfg["s_dt"]] if cfg["s_dt"] == cfg["av_dt"] else _NP_OF[FP]

    x = inp["x"].reshape(N, D).astype(np.float32)
    g1 = inp["ln1_g"].astype(np.float32)
    b1_ = inp["ln1_b"].astype(np.float32)
    g2 = inp["ln2_g"].astype(np.float32)
    b2_ = inp["ln2_b"].astype(np.float32)
    # [H, D, HS] -> [D, H*HS]
    Wq = np.transpose(inp["Wq"], (1, 0, 2)).reshape(D, D).astype(np.float32)
    Wk = np.transpose(inp["Wk"], (1, 0, 2)).reshape(D, D).astype(np.float32)
    Wv = np.transpose(inp["Wv"], (1, 0, 2)).reshape(D, D).astype(np.float32)
    scale = np.float32(D) ** -0.5
    Wq_eff = (g1[:, None] * Wq) * scale
    Wk_eff = g1[:, None] * Wk
    Wv_eff = g1[:, None] * Wv
    qbias = (b1_ @ Wq) * scale   # [512]
    kbias = b1_ @ Wk
    Wo = inp["Wo"].astype(np.float32)
    bo = (inp["bo"].astype(np.float32) + (b1_ @ Wv) @ Wo).astype(np.float32)
    Wg = inp["Wg"].astype(np.float32)
    bg = inp["bg"].astype(np.float32)
    Wn = inp["Wn"].astype(np.float32)
    bn_ = inp["bn"].astype(np.float32)
    wgn = np.zeros((D, 64), np.float32)
    wgn[:, 0:8] = g2[:, None] * Wg
    wgn[:, 32:40] = g2[:, None] * Wn
    bgn = np.zeros((64, 1), np.float32)
    bgn[0:8, 0] = bg + b2_ @ Wg
    bgn[32:40, 0] = bn_ + b2_ @ Wn
    masks = _build_masks(av_np)

    maps = []
    for c in range(8):
        b = c // 2
        hg = c % 2
        sl = slice(hg * 256, (hg + 1) * 256)
        own = _own_idx(c)
        maps.append({
            "xb": np.ascontiguousarray(x[b * T:(b + 1) * T]),
            "xown": np.ascontiguousarray(x[own]),
            "wq": np.ascontiguousarray(Wq_eff[:, sl]).astype(x_np),
            "wk": np.ascontiguousarray(Wk_eff[:, sl]).astype(x_np),
            "wv": np.ascontiguousarray(Wv_eff[:, sl]).astype(x_np),
            "qb": np.ascontiguousarray(qbias[sl].reshape(2, 128).T),
            "kb": np.ascontiguousarray(kbias[sl].reshape(2, 128).T),
            "wo": np.ascontiguousarray(Wo[sl, :]).astype(wo_np),
            "bo": bo,
            "masks": masks,
            "wgn": wgn,
            "bgn": bgn,
        })
    return maps


def host_route(noisy):
    """noisy: [N, E] f32. Returns idx_buf [E,CAP], gate_valid [E,CAP], nv [E]."""
    n_idx = np.arange(N)
    i1 = np.argmax(noisy, axis=1)
    v1 = noisy[n_idx, i1]
    masked = noisy.copy()
    masked[n_idx, i1] = -np.inf
    i2 = np.argmax(masked, axis=1)
    v2 = masked[n_idx, i2]
    # softmax over (v1, v2), v1 >= v2
    e2 = np.exp(v2 - v1)
    g1 = 1.0 / (1.0 + e2)
    g2 = e2 / (1.0 + e2)
    idx_buf = np.zeros((E, CAP), np.int32)
    gate_valid = np.zeros((E, CAP), np.float32)
    nv = np.zeros(E, np.int64)
    for e in range(E):
        sel = np.flatnonzero((i1 == e) | (i2 == e))
        kept = sel[:CAP]
        k = len(kept)
        nv[e] = k
        idx_buf[e, :k] = kept
        gate_valid[e, :k] = np.where(i1[kept] == e, g1[kept], g2[kept])
    return idx_buf, gate_valid, nv


def l2_in_maps(inp, h2_full, idx_buf, gate_valid, cfg=CFG):
    g2 = inp["ln2_g"].astype(np.float32)
    b2_ = inp["ln2_b"].astype(np.float32)
    W1 = np.asarray(inp["W1"], np.float32)
    b1 = np.asarray(inp["b1"], np.float32)
    W2 = np.asarray(inp["W2"], np.float32)
    b2 = np.asarray(inp["b2"], np.float32)
    maps = []
    for e in range(E):
        W1e = g2[:, None] * W1[e]
        b1e = b1[e] + b2_ @ W1[e]
        maps.append({
            "h2full": h2_full,
            "idxb": idx_buf[e][:, None],
            "gv": gate_valid[e][:, None],
            "w1": W1e.astype(NP_BF),
            "w2": W2[e].astype(NP_BF),
            "b1": b1e.astype(np.float32),
            "b2": b2[e].astype(np.float32),
        })
    return maps


def kernel(**inputs):
    inp = {k: np.asarray(v) for k, v in inputs.items()}
    l1, l2 = _get_built()

    maps1 = l1_in_maps(inp)
    r1 = run_bass_kernel_spmd(l1, maps1, core_ids=list(range(8))).results

    h1 = np.empty((N, D), np.float32)
    h2 = np.empty((N, D), _NP_OF[CFG["h2_dt"]])
    logits = np.empty((N, 8), np.float32)
    nlogits = np.empty((N, 8), np.float32)
    for c in range(8):
        own = _own_idx(c)
        h1[own] = r1[c]["h1o"]
        h2[own] = r1[c]["h2o"]
        logits[own] = r1[c]["lgt"][0:8].T
        nlogits[own] = r1[c]["lgt"][8:16].T
    noise = _get_noise()
    noisy = logits + noise * np.logaddexp(np.float32(0), nlogits).astype(np.float32)

    idx_buf, gate_valid, nv = host_route(noisy)

    maps2 = l2_in_maps(inp, h2, idx_buf, gate_valid)
    r2 = run_bass_kernel_spmd(l2, maps2, core_ids=list(range(8))).results

    out = h1
    for e in range(E):
        k = int(nv[e])
        out[idx_buf[e, :k]] += r2[e]["wout"][:k]
    return out.reshape(B, T, D)


# revision 48
# speedup vs baseline: 1.0039x; 1.0039x over previous
"""Trainium2 Bass kernel for nn_Block (attention + noisy top-2 MoE block).

Sharding (8 NeuronCores):
  Launch 1: core c -> batch b=c//2, head-group hg=c%2 (4 of 8 heads).
    LN1 -> q/k/v (local heads) -> causal softmax attention -> Wo partial,
    pairwise ReduceScatter over (hg0, hg1) -> each core owns 1024 tokens:
    residual + LN2 + noisy gating logits.
  Host: top-2 routing + capacity assignment (metadata only).
  Launch 2: core e -> expert e. Indirect-gather its CAP tokens, FFN
    (relu(x@W1+b1)@W2+b2) in bf16, scale by gate, return w_e.
  Host: out = h1; out[kept_e] += w_e  (indices unique within an expert).
"""

import numpy as np
import ml_dtypes

import concourse.bacc as bacc
import concourse.bass as bass
import concourse.mybir as mybir
import concourse.tile as tile
from concourse.bass_utils import run_bass_kernel_spmd
from concourse.masks import make_identity

B, T, D, H, HS = 4, 2048, 512, 8, 64
E, TOPK, CAP = 8, 2, 2048
N = B * T
EPS = 1e-5

FP = mybir.dt.float32
BF = mybir.dt.bfloat16
FR = mybir.dt.float32r
I32 = mybir.dt.int32
NP_BF = ml_dtypes.bfloat16

# dtype config: attention compute dtypes (routing-sensitive), FFN is bf16.
CFG = {
    "s_dt": FR,    # q/k and S=q@kT matmul dtype (f32r: fp32-precision, 1.5 cyc/row)
    "av_dt": FR,   # exp(S), v, AV matmul dtype
    "wo_dt": FR,   # attn^T / Wo matmul dtype
    "h2_dt": BF,   # h2 storage (expert gather table)
}

_NP_OF = {FP: np.float32, BF: NP_BF, FR: np.float32}


# --------------------------------------------------------------------------
# Launch 1: attention + gating logits
# --------------------------------------------------------------------------

def build_l1(cfg=CFG, no_collective=False):
    s_dt, av_dt, wo_dt, h2_dt = cfg["s_dt"], cfg["av_dt"], cfg["wo_dt"], cfg["h2_dt"]
    x_dt = s_dt if s_dt == av_dt else FP
    nc = bacc.Bacc(num_devices=8)

    xb = nc.dram_tensor("xb", [T, D], FP, kind="ExternalInput")
    xown = nc.dram_tensor("xown", [T // 2, D], FP, kind="ExternalInput")
    wq = nc.dram_tensor("wq", [D, 256], x_dt, kind="ExternalInput")
    wk = nc.dram_tensor("wk", [D, 256], x_dt, kind="ExternalInput")
    wv = nc.dram_tensor("wv", [D, 256], x_dt, kind="ExternalInput")
    qb = nc.dram_tensor("qb", [128, 2], FP, kind="ExternalInput")
    kb = nc.dram_tensor("kb", [128, 2], FP, kind="ExternalInput")
    wo = nc.dram_tensor("wo", [256, D], wo_dt, kind="ExternalInput")
    bo = nc.dram_tensor("bo", [D], FP, kind="ExternalInput")
    masks = nc.dram_tensor("masks", [128, 128], av_dt, kind="ExternalInput")
    wgn = nc.dram_tensor("wgn", [D, 64], FP, kind="ExternalInput")
    bgn = nc.dram_tensor("bgn", [64, 1], FP, kind="ExternalInput")

    h1o = nc.dram_tensor("h1o", [T // 2, D], FP, kind="ExternalOutput")
    h2o = nc.dram_tensor("h2o", [T // 2, D], h2_dt, kind="ExternalOutput")
    lgt = nc.dram_tensor("lgt", [16, T // 2], FP, kind="ExternalOutput")

    ccs = [nc.dram_tensor(f"cc{j}", [512, D], FP) for j in range(4)]
    rss = [nc.dram_tensor(f"rs{j}", [256, D], FP) for j in range(4)]

    with tile.TileContext(nc) as tc:
        with (
            tc.tile_pool(name="const", bufs=1) as const,
            tc.tile_pool(name="big", bufs=1) as big,
            tc.tile_pool(name="xtc", bufs=2) as xtc_pool,
            tc.tile_pool(name="qtc", bufs=2) as qtc_pool,
            tc.tile_pool(name="tmp", bufs=3) as tmp,
            tc.tile_pool(name="pexp", bufs=6) as pexp_pool,
            tc.tile_pool(name="rows", bufs=2) as rows,
            tc.tile_pool(name="epool", bufs=2) as epool,
            tc.tile_pool(name="small", bufs=3) as small,
            tc.tile_pool(name="psA", bufs=2, space="PSUM") as psA,
            tc.tile_pool(name="psS", bufs=3, space="PSUM") as psS,
            tc.tile_pool(name="psAV", bufs=1, space="PSUM") as psAV,
            tc.tile_pool(name="psT", bufs=1, space="PSUM") as psT,
        ):
            # ---- constants ----
            ident = const.tile([128, 128], FP)
            make_identity(nc, ident[:])
            tr_dt = BF if x_dt == BF else FP
            if tr_dt == FP:
                identx = ident
            else:
                identx = const.tile([128, 128], tr_dt)
                make_identity(nc, identx[:])
            zero = const.tile([128, 1], FP)
            nc.vector.memset(zero[:], 0.0)
            eps = const.tile([128, 1], FP)
            nc.vector.memset(eps[:], EPS)

            wq_sb = const.tile([128, 4, 256], x_dt)
            nc.scalar.dma_start(wq_sb[:], wq[:].rearrange("(ko p) m -> p ko m", p=128))
            wk_sb = const.tile([128, 4, 256], x_dt)
            nc.scalar.dma_start(wk_sb[:], wk[:].rearrange("(ko p) m -> p ko m", p=128))
            wv_sb = const.tile([128, 4, 256], x_dt)
            nc.scalar.dma_start(wv_sb[:], wv[:].rearrange("(ko p) m -> p ko m", p=128))
            wo_sb = const.tile([128, 2, 512], wo_dt)
            nc.scalar.dma_start(wo_sb[:], wo[:].rearrange("(ko p) m -> p ko m", p=128))
            wgn_sb = const.tile([128, 4, 64], FP)
            nc.scalar.dma_start(wgn_sb[:], wgn[:].rearrange("(ko p) m -> p ko m", p=128))
            bgn_sb = const.tile([64, 1], FP)
            nc.sync.dma_start(bgn_sb[:], bgn[:])
            qb_sb = const.tile([128, 2], FP)
            nc.sync.dma_start(qb_sb[:], qb[:])
            kb_sb = const.tile([128, 2], FP)
            nc.sync.dma_start(kb_sb[:], kb[:])

            def bcast_from_dram(dram, n):
                t = const.tile([128, n], FP, tag=f"bc_{dram.name}")
                src = bass.AP(tensor=dram, offset=0, ap=[[0, 128], [1, n]])
                nc.sync.dma_start(t[:], src)
                return t

            bo_bc = bcast_from_dram(bo, 512)

            masks_sb = const.tile([128, 128], av_dt)
            nc.scalar.dma_start(masks_sb[:], masks[:])

            # ---- persistent activations ----
            kT = big.tile([128, 2, T], s_dt)            # [hs-in-pair, head-pair, tok]
            v_aug = big.tile([128, 16, 4, 65], av_dt)   # [tok%128, tokblk, head, hs+1]
            onesf = const.tile([128, 16, 4, 1], FP)
            nc.vector.memset(onesf[:], 1.0)
            nc.vector.tensor_copy(v_aug[:, :, :, 64:65], onesf[:])

            # ---- LN1 stats prologue: ACT does all Sqrts before the Exps ----
            sqrt_insts = []
            last_exp = [None]
            mvs = const.tile([128, 16, 2], FP)
            rstds = const.tile([128, 16], FP)
            for j in range(4):
                xq = xtc_pool.tile([128, 4, 512], FP, tag="xq")
                nc.sync.dma_start(
                    xq[:], xb[j * 512:(j + 1) * 512, :].rearrange("(o p) m -> p o m", p=128))
                for t4 in range(4):
                    t = 4 * j + t4
                    stats = small.tile([128, 6], FP, tag="stats")
                    nc.vector.bn_stats(stats[:], xq[:, t4, :])
                    nc.vector.bn_aggr(mvs[:, t, :], stats[:])
                    sqrt_insts.append(nc.scalar.activation(
                        rstds[:, t:t + 1], mvs[:, t, 1:2],
                        mybir.ActivationFunctionType.Sqrt,
                        bias=eps[:], scale=1.0))
                    nc.vector.reciprocal(rstds[:, t:t + 1], rstds[:, t:t + 1])

            # ---- main loop over 512-token chunks ----
            for ch in range(4):
                # LN1 apply + x^T
                xTc = xtc_pool.tile([128, 4, 512], x_dt)
                xq2 = xtc_pool.tile([128, 4, 512], FP, tag="xq")
                nc.sync.dma_start(
                    xq2[:], xb[ch * 512:(ch + 1) * 512, :].rearrange("(o p) m -> p o m", p=128))
                for it4 in range(4):
                    t = ch * 4 + it4
                    lnx = tmp.tile([128, 512], FP, tag="w")
                    nc.vector.tensor_scalar(lnx[:], xq2[:, it4, :], scalar1=mvs[:, t, 0:1],
                                            scalar2=rstds[:, t:t + 1],
                                            op0=mybir.AluOpType.subtract,
                                            op1=mybir.AluOpType.mult)
                    if tr_dt != FP:
                        lnxc = tmp.tile([128, 512], tr_dt, tag="lnxb")
                        nc.vector.tensor_copy(lnxc[:], lnx[:])
                    else:
                        lnxc = lnx
                    pt = psT.tile([128, 512], tr_dt, tag="pt")
                    for db in range(4):
                        nc.tensor.transpose(pt[:, db * 128:(db + 1) * 128],
                                            lnxc[:, db * 128:(db + 1) * 128], identx[:])
                    nc.vector.tensor_copy(
                        xTc[:, :, it4 * 128:(it4 + 1) * 128],
                        pt[:].rearrange("p (a b) -> p a b", a=4))
                # q^T (chunk-local), k^T (persistent)
                qTc = qtc_pool.tile([128, 2, 512], s_dt)
                for hp in range(2):
                    pq = psA.tile([128, 512], FP, tag="pq", name="pq")
                    for db in range(4):
                        nc.tensor.matmul(pq[:], wq_sb[:, db, hp * 128:(hp + 1) * 128],
                                         xTc[:, db, :], start=db == 0, stop=db == 3)
                    nc.vector.tensor_scalar(qTc[:, hp, :], pq[:], scalar1=qb_sb[:, hp:hp + 1],
                                            scalar2=None, op0=mybir.AluOpType.add)
                    pk = psA.tile([128, 512], FP, tag="pq", name="pk")
                    for db in range(4):
                        nc.tensor.matmul(pk[:], wk_sb[:, db, hp * 128:(hp + 1) * 128],
                                         xTc[:, db, :], start=db == 0, stop=db == 3)
                    nc.vector.tensor_scalar(kT[:, hp, ch * 512:(ch + 1) * 512], pk[:],
                                            scalar1=kb_sb[:, hp:hp + 1],
                                            scalar2=None, op0=mybir.AluOpType.add)
                # v (row-major, augmented)
                for it4 in range(4):
                    t = ch * 4 + it4
                    pv = psA.tile([128, 512], FP, tag="pq", name="pv")
                    for db in range(4):
                        nc.tensor.matmul(pv[:, :256], xTc[:, db, it4 * 128:(it4 + 1) * 128],
                                         wv_sb[:, db, :], start=db == 0, stop=db == 3)
                    nc.vector.tensor_copy(
                        v_aug[:, t, :, 0:64],
                        pv[:, :256].rearrange("p (h e) -> p h e", h=4))
                # attention for this tq-chunk; heads of a pair row-packed on PE
                attnT = qtc_pool.tile([128, 2, 512], wo_dt, tag="attnT")
                for hp in range(2):
                    nvis = 4 * (ch + 1)
                    pavs = [psAV.tile([128, 512], FP, tag=f"pav{i}", name=f"pav{i}")
                            for i in range(2)]
                    for blk in range(nvis):
                        r = blk - 4 * ch
                        off = 128 * r if r > 0 else 0  # cols < off are fully masked
                        sss = []
                        for i in range(2):
                            base = i * 64
                            ss = psS.tile([128, 512], FP, tag="ss", name="ss")
                            nc.tensor.matmul(ss[:, off:],
                                             kT[base:base + 64, hp, blk * 128:(blk + 1) * 128],
                                             qTc[base:base + 64, hp, off:],
                                             start=True, stop=True,
                                             tile_position=(base, 0))
                            sss.append(ss)
                        pexps = []
                        for i in range(2):
                            pexp = pexp_pool.tile([128, 512], av_dt, tag="pexp", name="pexp")
                            exp_inst = nc.scalar.activation(
                                pexp[:, off:], sss[i][:, off:],
                                mybir.ActivationFunctionType.Exp,
                                bias=zero[:], scale=1.0)
                            last_exp[0] = exp_inst
                            if sqrt_insts is not None:
                                for si in sqrt_insts:
                                    tile.add_dep_helper(exp_inst.ins, si.ins, sync=False,
                                                        reason="act table grouping")
                                sqrt_insts = None
                            if r >= 0:
                                # only the 128-wide strip crossing the diagonal needs masking
                                nc.gpsimd.tensor_tensor(pexp[:, off:off + 128],
                                                        pexp[:, off:off + 128], masks_sb[:],
                                                        op=mybir.AluOpType.mult)
                            pexps.append(pexp)
                        for i in range(2):
                            nc.tensor.matmul(pavs[i][:65, off:], v_aug[:, blk, 2 * hp + i, :],
                                             pexps[i][:, off:],
                                             start=blk == 0, stop=blk == nvis - 1)
                    for i in range(2):
                        base = i * 64
                        rec = rows.tile([1, 512], FP, tag="rec", name="rec")
                        nc.vector.tensor_copy(rec[:], pavs[i][64:65, :])
                        nc.vector.reciprocal(rec[:], rec[:])
                        bc = rows.tile([64, 512], FP, tag="bc", name="bc")
                        nc.gpsimd.partition_broadcast(bc[:], rec[:])
                        nc.vector.tensor_tensor(attnT[base:base + 64, hp, :],
                                                pavs[i][0:64, :], bc[:],
                                                op=mybir.AluOpType.mult)
                # Wo for this tq-chunk + its pairwise reduce-scatter
                ho_big = qtc_pool.tile([128, 4, 512], FP, tag="hob", name="ho_big")
                for it4 in range(4):
                    po = psA.tile([128, 512], FP, tag="pq", name="po")
                    for kbk in range(2):
                        nc.tensor.matmul(po[:], attnT[:, kbk, it4 * 128:(it4 + 1) * 128],
                                         wo_sb[:, kbk, :], start=kbk == 0, stop=kbk == 1)
                    nc.vector.tensor_copy(ho_big[:, it4, :], po[:])
                nc.sync.dma_start(ccs[ch][:].rearrange("(o p) m -> p o m", p=128), ho_big[:])
                if no_collective:
                    half = big.tile([128, 2, 512], FP, tag="rshack", name="half")
                    nc.sync.dma_start(half[:], ccs[ch][:256, :].rearrange("(o p) m -> p o m", p=128))
                    nc.sync.dma_start(rss[ch][:].rearrange("(o p) m -> p o m", p=128), half[:])
                else:
                    nc.gpsimd.collective_compute(
                        "ReduceScatter", mybir.AluOpType.add,
                        replica_groups=[[0, 1], [2, 3], [4, 5], [6, 7]],
                        ins=[ccs[ch][:]], outs=[rss[ch][:]])

            # ---- stage E: residual + LN2 + h2^T ----
            h2T = big.tile([128, 4, T // 2], FP)
            for it in range(8):
                if it % 2 == 0:
                    rtb = epool.tile([128, 2, 512], FP, tag="rtb", name="rtb")
                    nc.sync.dma_start(
                        rtb[:], rss[it // 2][:].rearrange("(o p) m -> p o m", p=128))
                    xob = epool.tile([128, 2, 512], FP, tag="xob", name="xob")
                    nc.sync.dma_start(
                        xob[:], xown[it * 128:(it + 2) * 128, :].rearrange("(o p) m -> p o m", p=128))
                    h1st = epool.tile([128, 2, 512], FP, tag="h1st", name="h1st")
                    h2st = epool.tile([128, 2, 512], h2_dt, tag="h2st", name="h2st")
                h1t = h1st[:, it % 2, :]
                nc.gpsimd.tensor_tensor(h1t, rtb[:, it % 2, :], xob[:, it % 2, :],
                                        op=mybir.AluOpType.add)
                nc.gpsimd.tensor_tensor(h1t, h1t, bo_bc[:], op=mybir.AluOpType.add)
                stats = small.tile([128, 6], FP, tag="stats")
                nc.vector.bn_stats(stats[:], h1t)
                mv = small.tile([128, 2], FP, tag="mv")
                nc.vector.bn_aggr(mv[:], stats[:])
                rstd = small.tile([128, 1], FP, tag="rstd")
                e_sqrt = nc.scalar.activation(rstd[:], mv[:, 1:2],
                                              mybir.ActivationFunctionType.Sqrt,
                                              bias=eps[:], scale=1.0)
                if last_exp[0] is not None:
                    tile.add_dep_helper(e_sqrt.ins, last_exp[0].ins, sync=False,
                                        reason="act table grouping (ln2)")
                nc.vector.reciprocal(rstd[:], rstd[:])
                h2t = small.tile([128, 512], FP, tag="we")
                nc.vector.tensor_scalar(h2t[:], h1t, scalar1=mv[:, 0:1],
                                        scalar2=rstd[:],
                                        op0=mybir.AluOpType.subtract,
                                        op1=mybir.AluOpType.mult)
                nc.vector.tensor_copy(h2st[:, it % 2, :], h2t[:])
                pt2 = psT.tile([128, 512], FP, tag="pt")
                for db in range(4):
                    nc.tensor.transpose(pt2[:, db * 128:(db + 1) * 128],
                                        h2t[:, db * 128:(db + 1) * 128], ident[:])
                nc.vector.tensor_copy(
                    h2T[:, :, it * 128:(it + 1) * 128],
                    pt2[:].rearrange("p (a b) -> p a b", a=4))
                if it % 2 == 1:
                    j2 = it // 2
                    nc.sync.dma_start(
                        h1o[j2 * 256:(j2 + 1) * 256, :].rearrange("(o p) m -> p o m", p=128),
                        h1st[:])
                    nc.sync.dma_start(
                        h2o[j2 * 256:(j2 + 1) * 256, :].rearrange("(o p) m -> p o m", p=128),
                        h2st[:])

            # ---- stage F: gating (rows 0-7 logits, 32-39 noise logits) ----
            for c2 in range(2):
                pg = psA.tile([128, 512], FP, tag="pq", name="pg")
                for db in range(4):
                    nc.tensor.matmul(pg[:64, :], wgn_sb[:, db, :],
                                     h2T[:, db, c2 * 512:(c2 + 1) * 512],
                                     start=db == 0, stop=db == 3)
                nc.vector.tensor_scalar(pg[:64, :], pg[:64, :], scalar1=bgn_sb[:],
                                        scalar2=None, op0=mybir.AluOpType.add)
                t1 = rows.tile([8, 512], FP, tag="sp")
                nc.vector.tensor_copy(t1[:], pg[0:8, :])
                nc.sync.dma_start(lgt[0:8, c2 * 512:(c2 + 1) * 512], t1[:])
                t2 = rows.tile([8, 512], FP, tag="sp")
                nc.vector.tensor_copy(t2[:], pg[32:40, :])
                nc.sync.dma_start(lgt[8:16, c2 * 512:(c2 + 1) * 512], t2[:])

    nc.compile()
    return nc


# --------------------------------------------------------------------------
# Launch 2: expert FFN
# --------------------------------------------------------------------------

def build_l2(cfg=CFG):
    h2_dt = cfg["h2_dt"]
    nc = bacc.Bacc(num_devices=8)

    h2full = nc.dram_tensor("h2full", [N, D], h2_dt, kind="ExternalInput")
    idxb = nc.dram_tensor("idxb", [CAP, 1], I32, kind="ExternalInput")
    gv = nc.dram_tensor("gv", [CAP, 1], FP, kind="ExternalInput")
    w1 = nc.dram_tensor("w1", [D, 4 * D], BF, kind="ExternalInput")
    w2 = nc.dram_tensor("w2", [4 * D, D], BF, kind="ExternalInput")
    b1 = nc.dram_tensor("b1", [4 * D], FP, kind="ExternalInput")
    b2 = nc.dram_tensor("b2", [D], FP, kind="ExternalInput")
    wout = nc.dram_tensor("wout", [CAP, D], FP, kind="ExternalOutput")

    with tile.TileContext(nc) as tc:
        with (
            tc.tile_pool(name="const", bufs=1) as const,
            tc.tile_pool(name="chunk", bufs=2) as chunk,
            tc.tile_pool(name="tmp", bufs=3) as tmp,
            tc.tile_pool(name="psM", bufs=3, space="PSUM") as psM,
            tc.tile_pool(name="psO", bufs=2, space="PSUM") as psO,
        ):
            w1_sb = const.tile([128, 4, 4 * D], BF)
            nc.sync.dma_start(w1_sb[:], w1[:].rearrange("(ko p) m -> p ko m", p=128))
            w2_sb = const.tile([128, 16, D], BF)
            nc.scalar.dma_start(w2_sb[:], w2[:].rearrange("(ko p) m -> p ko m", p=128))
            idx_sb = const.tile([128, 16], I32)
            nc.sync.dma_start(idx_sb[:], idxb[:, 0].rearrange("(o p) -> p o", p=128))
            gv_sb = const.tile([128, 16], FP)
            nc.sync.dma_start(gv_sb[:], gv[:, 0].rearrange("(o p) -> p o", p=128))
            b1_sb = const.tile([128, 16], FP)
            nc.sync.dma_start(b1_sb[:], b1[:].rearrange("(o p) -> p o", p=128))
            b2_bc = const.tile([128, 512], FP)
            nc.sync.dma_start(b2_bc[:], bass.AP(tensor=b2, offset=0, ap=[[0, 128], [1, 512]]))

            for tc4 in range(4):  # 512-token chunks
                xgT = chunk.tile([128, 4, 512], h2_dt, tag="xgT")
                for it4 in range(4):
                    t = tc4 * 4 + it4
                    xg = tmp.tile([128, 512], h2_dt, tag="xg")
                    nc.gpsimd.indirect_dma_start(
                        out=xg[:], out_offset=None, in_=h2full[:],
                        in_offset=bass.IndirectOffsetOnAxis(ap=idx_sb[:, t:t + 1], axis=0))
                    for db in range(4):
                        eng = nc.sync if db % 2 == 0 else nc.scalar
                        eng.dma_start_transpose(
                            xgT[:, db, it4 * 128:(it4 + 1) * 128],
                            xg[:, db * 128:(db + 1) * 128])
                midT = chunk.tile([128, 16, 512], BF, tag="midT")
                for fb in range(16):
                    pm = psM.tile([128, 512], FP, tag="pm")
                    for db in range(4):
                        nc.tensor.matmul(pm[:], w1_sb[:, db, fb * 128:(fb + 1) * 128],
                                         xgT[:, db, :], start=db == 0, stop=db == 3)
                    nc.scalar.activation(midT[:, fb, :], pm[:],
                                         mybir.ActivationFunctionType.Relu,
                                         bias=b1_sb[:, fb:fb + 1], scale=1.0)
                ost = chunk.tile([128, 4, 512], FP, tag="ost")
                for it4 in range(4):
                    t = tc4 * 4 + it4
                    po = psO.tile([128, 512], FP, tag="po")
                    for fb in range(16):
                        nc.tensor.matmul(po[:], midT[:, fb, it4 * 128:(it4 + 1) * 128],
                                         w2_sb[:, fb, :], start=fb == 0, stop=fb == 15)
                    ot = ost[:, it4, :]
                    nc.vector.tensor_tensor(ot, po[:], b2_bc[:], op=mybir.AluOpType.add)
                    nc.vector.tensor_scalar(ot, ot, scalar1=gv_sb[:, t:t + 1], scalar2=None,
                                            op0=mybir.AluOpType.mult)
                nc.sync.dma_start(
                    wout[tc4 * 512:(tc4 + 1) * 512, :].rearrange("(o p) m -> p o m", p=128),
                    ost[:])

    nc.compile()
    return nc


# --------------------------------------------------------------------------
# Host glue
# --------------------------------------------------------------------------

_BUILT = {}


def _own_idx(c):
    """Global token indices owned by core c, in its local row order."""
    b, hg = c // 2, c % 2
    return np.concatenate(
        [b * T + 512 * j + 256 * hg + np.arange(256) for j in range(4)])


def _get_built():
    if "l1" not in _BUILT:
        _BUILT["l1"] = build_l1()
        _BUILT["l2"] = build_l2()
    return _BUILT["l1"], _BUILT["l2"]


def _get_noise():
    # The reference adds jax.random.normal(key(42), [B,T,E]) noise to the
    # gating logits; threefry is deterministic, so precompute it host-side.
    if "noise" not in _BUILT:
        import jax
        import jax.numpy as jnp
        try:
            dev = jax.devices("cpu")[0]
        except Exception:
            dev = None
        if dev is not None:
            with jax.default_device(dev):
                noise = jax.random.normal(jax.random.key(42), (B, T, E), jnp.float32)
        else:
            noise = jax.random.normal(jax.random.key(42), (B, T, E), jnp.float32)
        _BUILT["noise"] = np.asarray(noise).reshape(N, E)
    return _BUILT["noise"]


def _build_masks(np_dt):
    j = np.arange(128)[None, :]
    p = np.arange(128)[:, None]
    return (j >= p).astype(np_dt)


def l1_in_maps(inp, cfg=CFG):
    s_np = _NP_OF[cfg["s_dt"]]
    av_np = _NP_OF[cfg["av_dt"]]
    wo_np = _NP_OF[cfg["wo_dt"]]
    x_np = _NP_OF[cfg["s_dt"]] if cfg["s_dt"] == cfg["av_dt"] else _NP_OF[FP]

    x = inp["x"].reshape(N, D).astype(np.float32)
    g1 = inp["ln1_g"].astype(np.float32)
    b1_ = inp["ln1_b"].astype(np.float32)
    g2 = inp["ln2_g"].astype(np.float32)
    b2_ = inp["ln2_b"].astype(np.float32)
    # [H, D, HS] -> [D, H*HS]
    Wq = np.transpose(inp["Wq"], (1, 0, 2)).reshape(D, D).astype(np.float32)
    Wk = np.transpose(inp["Wk"], (1, 0, 2)).reshape(D, D).astype(np.float32)
    Wv = np.transpose(inp["Wv"], (1, 0, 2)).reshape(D, D).astype(np.float32)
    scale = np.float32(D) ** -0.5
    Wq_eff = (g1[:, None] * Wq) * scale
    Wk_eff = g1[:, None] * Wk
    Wv_eff = g1[:, None] * Wv
    qbias = (b1_ @ Wq) * scale   # [512]
    kbias = b1_ @ Wk
    Wo = inp["Wo"].astype(np.float32)
    bo = (inp["bo"].astype(np.float32) + (b1_ @ Wv) @ Wo).astype(np.float32)
    Wg = inp["Wg"].astype(np.float32)
    bg = inp["bg"].astype(np.float32)
    Wn = inp["Wn"].astype(np.float32)
    bn_ = inp["bn"].astype(np.float32)
    wgn = np.zeros((D, 64), np.float32)
    wgn[:, 0:8] = g2[:, None] * Wg
    wgn[:, 32:40] = g2[:, None] * Wn
    bgn = np.zeros((64, 1), np.float32)
    bgn[0:8, 0] = bg + b2_ @ Wg
    bgn[32:40, 0] = bn_ + b2_ @ Wn
    masks = _build_masks(av_np)

    maps = []
    for c in range(8):
        b = c // 2
        hg = c % 2
        sl = slice(hg * 256, (hg + 1) * 256)
        own = _own_idx(c)
        maps.append({
            "xb": np.ascontiguousarray(x[b * T:(b + 1) * T]),
            "xown": np.ascontiguousarray(x[own]),
            "wq": np.ascontiguousarray(Wq_eff[:, sl]).astype(x_np),
            "wk": np.ascontiguousarray(Wk_eff[:, sl]).astype(x_np),
            "wv": np.ascontiguousarray(Wv_eff[:, sl]).astype(x_np),
            "qb": np.ascontiguousarray(qbias[sl].reshape(2, 128).T),
            "kb": np.ascontiguousarray(kbias[sl].reshape(2, 128).T),
            "wo": np.ascontiguousarray(Wo[sl, :]).astype(wo_np),
            "bo": bo,
            "masks": masks,
            "wgn": wgn,
            "bgn": bgn,
        })
    return maps


def host_route(noisy):
    """noisy: [N, E] f32. Returns idx_buf [E,CAP], gate_valid [E,CAP], nv [E]."""
    n_idx = np.arange(N)
    i1 = np.argmax(noisy, axis=1)
    v1 = noisy[n_idx, i1]
    masked = noisy.copy()
    masked[n_idx, i1] = -np.inf
    i2 = np.argmax(masked, axis=1)
    v2 = masked[n_idx, i2]
    # softmax over (v1, v2), v1 >= v2
    e2 = np.exp(v2 - v1)
    g1 = 1.0 / (1.0 + e2)
    g2 = e2 / (1.0 + e2)
    idx_buf = np.zeros((E, CAP), np.int32)
    gate_valid = np.zeros((E, CAP), np.float32)
    nv = np.zeros(E, np.int64)
    for e in range(E):
        sel = np.flatnonzero((i1 == e) | (i2 == e))
        kept = sel[:CAP]
        k = len(kept)
        nv[e] = k
        idx_buf[e, :k] = kept
        gate_valid[e, :k] = np.where(i1[kept] == e, g1[kept], g2[kept])
    return idx_buf, gate_valid, nv


def l2_in_maps(inp, h2_full, idx_buf, gate_valid, cfg=CFG):
    g2 = inp["ln2_g"].astype(np.float32)
    b2_ = inp["ln2_b"].astype(np.float32)
    W1 = np.asarray(inp["W1"], np.float32)
    b1 = np.asarray(inp["b1"], np.float32)
    W2 = np.asarray(inp["W2"], np.float32)
    b2 = np.asarray(inp["b2"], np.float32)
    maps = []
    for e in range(E):
        W1e = g2[:, None] * W1[e]
        b1e = b1[e] + b2_ @ W1[e]
        maps.append({
            "h2full": h2_full,
            "idxb": idx_buf[e][:, None],
            "gv": gate_valid[e][:, None],
            "w1": W1e.astype(NP_BF),
            "w2": W2[e].astype(NP_BF),
            "b1": b1e.astype(np.float32),
            "b2": b2[e].astype(np.float32),
        })
    return maps


def kernel(**inputs):
    inp = {k: np.asarray(v) for k, v in inputs.items()}
    l1, l2 = _get_built()

    maps1 = l1_in_maps(inp)
    r1 = run_bass_kernel_spmd(l1, maps1, core_ids=list(range(8))).results

    h1 = np.empty((N, D), np.float32)
    h2 = np.empty((N, D), _NP_OF[CFG["h2_dt"]])
    logits = np.empty((N, 8), np.float32)
    nlogits = np.empty((N, 8), np.float32)
    for c in range(8):
        own = _own_idx(c)
        h1[own] = r1[c]["h1o"]
        h2[own] = r1[c]["h2o"]
        logits[own] = r1[c]["lgt"][0:8].T
        nlogits[own] = r1[c]["lgt"][8:16].T
    noise = _get_noise()
    noisy = logits + noise * np.logaddexp(np.float32(0), nlogits).astype(np.float32)

    idx_buf, gate_valid, nv = host_route(noisy)

    maps2 = l2_in_maps(inp, h2, idx_buf, gate_valid)
    r2 = run_bass_kernel_spmd(l2, maps2, core_ids=list(range(8))).results

    out = h1
    for e in range(E):
        k = int(nv[e])
        out[idx_buf[e, :k]] += r2[e]["wout"][:k]
    return out.reshape(B, T, D)
